# revision 1
# baseline (speedup 1.0000x reference)
"""Expert-parallel CMoE kernel for 8 Trainium2 NeuronCores.

Sharding (hardcoded for B=8, T=2048, D=1024, F=2048, E=16, C=1024):
  core k owns batch k (token shift, receptance, output) and experts
  {2k, 2k+1} (FFN). Hash routing is int math on token_ids, done on host;
  the resulting permutations ship to the cores as index tensors.

Device dataflow per core:
  phase A: token shift in bf16, scatter packed xk rows into the dispatch
           buffers. The dispatch AllToAll is split in two (tokens from
           the first / second half of each batch) so the first collective
           fires mid-phase-A and overlaps the rest of it. The receptance
           r = sigmoid(xr @ w_recept.T) is interleaved into the loop per
           512 tokens, so the PE warms up while dispatch is in flight.
           Loads go on the sync HWDGE queue and stores on the scalar
           HWDGE queue so the load stream never stalls behind compute.
  phase C: per expert: transposing dma_gather (rows -> feature-major)
           -> FFN1 -> relu^2 -> FFN2 (token-major out, weights moving)
           -> indirect scatter into the combine buffers; one AllToAll
           per expert, the first overlapping the second expert's FFN.
  phase D: gather own tokens' y rows, multiply by r, write fp32 output.
All matmuls bf16 with fp32 PSUM accumulation; dropped tokens and empty
expert slots are routed through zeroed trash rows.
"""
import sys

for _p in ("/opt/trn_rl_repo", "/root/.axon_site/_ro/trn_rl_repo"):
    if _p not in sys.path:
        sys.path.append(_p)

import numpy as np
import ml_dtypes

import concourse.bass as bass
import concourse.bacc as bacc
import concourse.mybir as mybir
import concourse.tile as tile
from concourse.bass_utils import run_bass_kernel_spmd

P = 128
B, T, D, F, E = 8, 2048, 1024, 2048, 16
N = B * T
C = max(4, N // E)          # 1024
HALF = 512                  # pos-split for the two dispatch collectives
HASH_PRIME = 5099
NCORES = 8
EPC = E // NCORES           # experts per core = 2
BF16 = mybir.dt.bfloat16
F32 = mybir.dt.float32
I16 = mybir.dt.int16
I32 = mybir.dt.int32
nbf16 = ml_dtypes.bfloat16
AF = mybir.ActivationFunctionType

_CACHE = {}


def _r16(v):
    return int(-(-int(v) // 16) * 16)


def _wrap16(a):
    a = np.asarray(a, np.int16)
    w = a.reshape(-1, 16).T.copy()       # j at [j%16, j//16]
    return np.tile(w, (8, 1))            # replicated across 8 Q7 cores


def _route(token_ids):
    tid = np.asarray(token_ids).reshape(N).astype(np.int64)
    e = (tid * HASH_PRIME) % E
    onehot = (e[:, None] == np.arange(E)).astype(np.int64)
    pos = onehot.cumsum(0)[np.arange(N), e] - 1
    keep = pos < C
    return e, pos, keep


def _build_indices(token_ids):
    e, pos, keep = _route(token_ids)
    src = np.arange(N) // T
    dst = e // EPC
    el = e % EPC

    # ---- dispatch: two chunks split by local token index (first/second
    # half of each batch) so the first collective can fire mid-phase-A
    local_t = np.arange(N) % T
    in_a = keep & (local_t < T // 2)
    in_b = keep & (local_t >= T // 2)

    def pack(mask):
        rank = np.zeros(N, np.int64)
        cnt = np.zeros((NCORES, NCORES), np.int64)
        for n in np.nonzero(mask)[0]:
            rank[n] = cnt[src[n], dst[n]]
            cnt[src[n], dst[n]] += 1
        return rank, _r16(cnt.max())

    rank_a, Ka = pack(in_a)
    rank_b, Kb = pack(in_b)
    srcA = np.where(in_a, dst * Ka + rank_a, NCORES * Ka)
    # chunk-b rows live at offset NCORES*Ka in the combined recv1 tile
    srcB = np.where(in_b, dst * Kb + rank_b, NCORES * Kb)
    Ta = T // 2 // P - 1                 # last tile writing chunk a

    # recv-side: slot (core d, el, c) -> row in combined recv1
    # [0 : 8Ka) chunk a, [8Ka : 8Ka+8Kb) chunk b, last row zero
    ZR1 = NCORES * (Ka + Kb)
    recv_row = np.full((NCORES, EPC * C), ZR1, np.int64)
    for n in np.nonzero(in_a)[0]:
        recv_row[dst[n], el[n] * C + pos[n]] = src[n] * Ka + rank_a[n]
    for n in np.nonzero(in_b)[0]:
        recv_row[dst[n], el[n] * C + pos[n]] = \
            NCORES * Ka + src[n] * Kb + rank_b[n]

    # ---- combine: one chunk per expert parity (el)
    rank_0, K0 = pack(keep & (el == 0))
    rank_1, K1 = pack(keep & (el == 1))
    ZROW = NCORES * (K0 + K1)
    sl2 = np.zeros((NCORES, EPC, C), np.int64)
    sl2[:, 0, :] = NCORES * K0
    sl2[:, 1, :] = NCORES * K1
    ygather = np.full(N, ZROW, np.int64)
    for n in range(N):
        if not keep[n]:
            continue
        if el[n] == 0:
            sl2[dst[n], 0, pos[n]] = src[n] * K0 + rank_0[n]
            ygather[n] = dst[n] * K0 + rank_0[n]
        else:
            sl2[dst[n], 1, pos[n]] = src[n] * K1 + rank_1[n]
            ygather[n] = NCORES * K0 + dst[n] * K1 + rank_1[n]

    per_core = []
    for k in range(NCORES):
        tok = slice(k * T, (k + 1) * T)
        # recv gather indices laid out [el, ck-half] -> flat [2*HALF]
        per_core.append({
            "srcA32": srcA[tok].astype(np.int32).reshape(T // P, P).T.copy(),
            "srcB32": srcB[tok].astype(np.int32).reshape(T // P, P).T.copy(),
            "slot16": _wrap16(recv_row[k]),
            "sl2_32": np.concatenate(
                [sl2[k, 0].reshape(-1, P).T, sl2[k, 1].reshape(-1, P).T],
                axis=1).astype(np.int32).copy(),
            "ygather16": _wrap16(ygather[tok]),
        })
    return (Ka, Kb, K0, K1, Ta), per_core


def _build_nc(cfg):
    Ka, Kb, K0, K1, Ta = cfg
    K2 = {0: K0, 1: K1}
    off = {0: 0, 1: NCORES * K0}
    R2 = NCORES * (K0 + K1)
    nc = bacc.Bacc("TRN2", target_bir_lowering=False, debug=False,
                   num_devices=NCORES)

    x_ext = nc.dram_tensor("x_ext", [T + 1, D], F32, kind="ExternalInput")
    maa_k = nc.dram_tensor("maa_k", [1, D], BF16, kind="ExternalInput")
    maa_r = nc.dram_tensor("maa_r", [1, D], BF16, kind="ExternalInput")
    wrt = nc.dram_tensor("wrt", [D, D], BF16, kind="ExternalInput")
    wk = nc.dram_tensor("wk", [EPC, D, F], BF16, kind="ExternalInput")
    wv = nc.dram_tensor("wv", [EPC, F, D], BF16, kind="ExternalInput")
    srcA32 = nc.dram_tensor("srcA32", [P, T // P], I32, kind="ExternalInput")
    srcB32 = nc.dram_tensor("srcB32", [P, T // P], I32, kind="ExternalInput")
    slot16 = nc.dram_tensor("slot16", [P, EPC * C // 16], I16,
                            kind="ExternalInput")
    sl2_32 = nc.dram_tensor("sl2_32", [P, EPC * C // P], I32,
                            kind="ExternalInput")
    ygather16 = nc.dram_tensor("ygather16", [P, T // 16], I16,
                               kind="ExternalInput")
    iota16 = nc.dram_tensor("iota16", [P, T // 16], I16, kind="ExternalInput")
    out = nc.dram_tensor("out", [T, D], F32, kind="ExternalOutput")

    DC = D // P          # 8
    FC = F // P          # 16
    rg = [list(range(NCORES))]

    with tile.TileContext(nc) as tc:
        with (
            tc.tile_pool(name="dram", bufs=1, space="DRAM") as dram,
            tc.tile_pool(name="misc", bufs=1) as misc,
            tc.tile_pool(name="psr", bufs=1, space="PSUM") as psr,
            tc.tile_pool(name="psh", bufs=2, space="PSUM") as psh,
            tc.tile_pool(name="psy", bufs=2, space="PSUM") as psy,
        ):
            a1a_in = dram.tile([NCORES * Ka + 1, D], BF16)
            a1b_in = dram.tile([NCORES * Kb + 1, D], BF16)
            recv1 = dram.tile([NCORES * (Ka + Kb) + 1, D], BF16)
            a2_in = {eli: dram.tile([NCORES * K2[eli] + 1, D], BF16,
                                    name=f"a2in_{eli}")
                     for eli in range(EPC)}
            recv2 = dram.tile([R2 + 1, D], BF16)
            xr_bufs = [dram.tile([512, D], BF16, name=f"xr_buf{i}")
                       for i in range(4)]
            r_buf = dram.tile([T, D], BF16)

            zrow = misc.tile([1, D], BF16)
            nc.vector.memzero(zrow[:])
            nc.sync.dma_start(
                out=recv1[NCORES * (Ka + Kb):NCORES * (Ka + Kb) + 1, :],
                in_=zrow[:])
            nc.sync.dma_start(out=recv2[R2:R2 + 1, :], in_=zrow[:])

            maakb = misc.tile([P, D], BF16)
            nc.sync.dma_start(out=maakb[:], in_=maa_k[:].to_broadcast([P, D]))
            maarb = misc.tile([P, D], BF16)
            nc.sync.dma_start(out=maarb[:], in_=maa_r[:].to_broadcast([P, D]))

            sA32 = misc.tile([P, T // P], I32)
            nc.sync.dma_start(out=sA32[:], in_=srcA32[:])
            sB32 = misc.tile([P, T // P], I32)
            nc.sync.dma_start(out=sB32[:], in_=srcB32[:])
            sl16 = misc.tile([P, EPC * C // 16], I16)
            nc.sync.dma_start(out=sl16[:], in_=slot16[:])
            s232 = misc.tile([P, EPC * C // P], I32)
            nc.sync.dma_start(out=s232[:], in_=sl2_32[:])
            yg16 = misc.tile([P, T // 16], I16)
            nc.sync.dma_start(out=yg16[:], in_=ygather16[:])
            io16 = misc.tile([P, T // 16], I16)
            nc.sync.dma_start(out=io16[:], in_=iota16[:])

            wrt_sb = misc.tile([P, DC, D], BF16)
            nc.sync.dma_start(out=wrt_sb[:],
                              in_=wrt.rearrange("(c p) e -> p c e", p=P))

            # ---- phase A (token shift) with receptance interleaved.
            # loads on sync HWDGE; stores on scalar HWDGE so the sync queue
            # streams loads ahead instead of waiting on compute.
            with (
                tc.tile_pool(name="pa", bufs=4) as pa,
                tc.tile_pool(name="prx", bufs=3) as prx,
            ):
                for t in range(T // P):
                    xc = pa.tile([P, D], F32, tag="xc")
                    nc.sync.dma_start(out=xc[:],
                                      in_=x_ext[1 + t * P:1 + (t + 1) * P, :])
                    xp = pa.tile([P, D], F32, tag="xp")
                    nc.sync.dma_start(out=xp[:], in_=x_ext[t * P:(t + 1) * P, :])
                    xcb = pa.tile([P, D], BF16, tag="xcb")
                    nc.scalar.activation(out=xcb[:], in_=xc[:], func=AF.Copy)
                    dx = pa.tile([P, D], BF16, tag="dx")
                    nc.vector.tensor_sub(out=dx[:], in0=xp[:], in1=xc[:])
                    tmp = pa.tile([P, D], BF16, tag="tmp")
                    xr = pa.tile([P, D], BF16, tag="xr")
                    nc.vector.tensor_mul(out=tmp[:], in0=dx[:], in1=maarb[:])
                    nc.vector.tensor_add(out=xr[:], in0=tmp[:], in1=xcb[:])
                    nc.scalar.dma_start(
                        out=xr_bufs[t // 4][(t % 4) * P:(t % 4 + 1) * P, :],
                        in_=xr[:])
                    tmp2 = pa.tile([P, D], BF16, tag="tmp2")
                    xk = pa.tile([P, D], BF16, tag="xk")
                    nc.vector.tensor_mul(out=tmp2[:], in0=dx[:], in1=maakb[:])
                    nc.vector.tensor_add(out=xk[:], in0=tmp2[:], in1=xcb[:])
                    if t <= Ta:
                        nc.gpsimd.indirect_dma_start(
                            out=a1a_in[:],
                            out_offset=bass.IndirectOffsetOnAxis(
                                ap=sA32[:, t:t + 1], axis=0),
                            in_=xk[:], in_offset=None)
                    nc.gpsimd.indirect_dma_start(
                        out=a1b_in[:],
                        out_offset=bass.IndirectOffsetOnAxis(
                            ap=sB32[:, t:t + 1], axis=0),
                        in_=xk[:], in_offset=None)
                    if t == Ta:
                        nc.gpsimd.collective_compute(
                            "AllToAll", mybir.AluOpType.bypass,
                            replica_groups=rg,
                            ins=[a1a_in[0:NCORES * Ka, :]],
                            outs=[recv1[0:NCORES * Ka, :]])
                    if t % 4 == 3:
                        # receptance for the 512 tokens just shifted
                        ck = t // 4
                        xrT = prx.tile([P, DC, 512], BF16, tag="xrT")
                        nc.gpsimd.dma_gather(
                            out_ap=xrT[:], in_ap=xr_bufs[ck][:],
                            idxs_ap=io16[:, 0:32],
                            num_idxs=512, num_idxs_reg=512, elem_size=D,
                            transpose=True)
                        for tt in range(4):
                            pr0 = psr.tile([P, 512], F32, space="PSUM", tag="pr0")
                            pr1 = psr.tile([P, 512], F32, space="PSUM", tag="pr1")
                            for dc in range(DC):
                                nc.tensor.matmul(
                                    out=pr0[:],
                                    lhsT=xrT[:, dc, tt * P:(tt + 1) * P],
                                    rhs=wrt_sb[:, dc, 0:512],
                                    start=(dc == 0), stop=(dc == DC - 1))
                                nc.tensor.matmul(
                                    out=pr1[:],
                                    lhsT=xrT[:, dc, tt * P:(tt + 1) * P],
                                    rhs=wrt_sb[:, dc, 512:1024],
                                    start=(dc == 0), stop=(dc == DC - 1))
                            rsb = prx.tile([P, D], BF16, tag="rsb")
                            nc.scalar.activation(out=rsb[:, 0:512], in_=pr0[:],
                                                 func=AF.Sigmoid)
                            nc.scalar.activation(out=rsb[:, 512:1024],
                                                 in_=pr1[:], func=AF.Sigmoid)
                            r0 = ck * 512 + tt * P
                            nc.scalar.dma_start(out=r_buf[r0:r0 + P, :],
                                                in_=rsb[:])

            nc.gpsimd.collective_compute(
                "AllToAll", mybir.AluOpType.bypass, replica_groups=rg,
                ins=[a1b_in[0:NCORES * Kb, :]],
                outs=[recv1[NCORES * Ka:NCORES * (Ka + Kb), :]])

            # ---------------- phase C: expert FFNs
            with (
                tc.tile_pool(name="pwk", bufs=2) as pwk,
                tc.tile_pool(name="pwv", bufs=1) as pwv,
                tc.tile_pool(name="pfx", bufs=2) as pfx,
                tc.tile_pool(name="pfh", bufs=2) as pfh,
                tc.tile_pool(name="pfy", bufs=2) as pfy,
            ):
                for el in range(EPC):
                    wk_sb = pwk.tile([P, DC, F], BF16, tag="wk")
                    nc.sync.dma_start(
                        out=wk_sb[:],
                        in_=wk[el].rearrange("(c p) f -> p c f", p=P))
                    wv_sb = pwv.tile([P, FC, D], BF16, tag="wv")
                    nc.sync.dma_start(
                        out=wv_sb[:],
                        in_=wv[el].rearrange("(c p) f -> p c f", p=P))
                    for ck in range(2):
                        XT = pfx.tile([P, DC, 512], BF16, tag="XT")
                        col0 = (el * C + ck * 512) // 16
                        nc.gpsimd.dma_gather(
                            out_ap=XT[:], in_ap=recv1[:],
                            idxs_ap=sl16[:, col0:col0 + 32],
                            num_idxs=512, num_idxs_reg=512, elem_size=D,
                            transpose=True)
                        ht = pfh.tile([P, FC, 512], BF16, tag="ht")
                        for ft in range(FC):
                            ph = psh.tile([P, 512], F32, space="PSUM", tag="ph")
                            for dc in range(DC):
                                nc.tensor.matmul(
                                    out=ph[:],
                                    lhsT=wk_sb[:, dc, ft * P:(ft + 1) * P],
                                    rhs=XT[:, dc, :],
                                    start=(dc == 0), stop=(dc == DC - 1))
                            hr = pfh.tile([P, 512], BF16, tag="hr")
                            nc.scalar.activation(out=hr[:], in_=ph[:],
                                                 func=AF.Relu)
                            nc.vector.tensor_mul(out=ht[:, ft, :], in0=hr[:],
                                                 in1=hr[:])
                        for tt in range(4):
                            ysb = pfy.tile([P, D], BF16, tag="ysb")
                            py0 = psy.tile([P, 512], F32, space="PSUM", tag="py0")
                            py1 = psy.tile([P, 512], F32, space="PSUM", tag="py1")
                            for fc in range(FC):
                                nc.tensor.matmul(
                                    out=py0[:],
                                    lhsT=ht[:, fc, tt * P:(tt + 1) * P],
                                    rhs=wv_sb[:, fc, 0:512],
                                    start=(fc == 0), stop=(fc == FC - 1))
                                nc.tensor.matmul(
                                    out=py1[:],
                                    lhsT=ht[:, fc, tt * P:(tt + 1) * P],
                                    rhs=wv_sb[:, fc, 512:1024],
                                    start=(fc == 0), stop=(fc == FC - 1))
                            nc.vector.tensor_copy(out=ysb[:, 0:512], in_=py0[:])
                            nc.vector.tensor_copy(out=ysb[:, 512:1024],
                                                  in_=py1[:])
                            scol = el * (C // P) + ck * 4 + tt
                            nc.gpsimd.indirect_dma_start(
                                out=a2_in[el][:],
                                out_offset=bass.IndirectOffsetOnAxis(
                                    ap=s232[:, scol:scol + 1], axis=0),
                                in_=ysb[:], in_offset=None)
                    nc.gpsimd.collective_compute(
                        "AllToAll", mybir.AluOpType.bypass, replica_groups=rg,
                        ins=[a2_in[el][0:NCORES * K2[el], :]],
                        outs=[recv2[off[el]:off[el] + NCORES * K2[el], :]])

            # ---------------- phase D: gather own rows, multiply by r
            with tc.tile_pool(name="pd", bufs=4) as pd:
                for ck in range(T // 512):
                    yg = pd.tile([P, 4, D], BF16, tag="yg")
                    nc.gpsimd.dma_gather(
                        out_ap=yg[:], in_ap=recv2[:],
                        idxs_ap=yg16[:, ck * 32:(ck + 1) * 32],
                        num_idxs=512, num_idxs_reg=512, elem_size=D,
                        transpose=False)
                    rw = pd.tile([P, 4, D], BF16, tag="rw")
                    nc.sync.dma_start(
                        out=rw[:],
                        in_=r_buf[ck * 512:(ck + 1) * 512, :].rearrange(
                            "(a p) d -> p a d", p=P))
                    yo = pd.tile([P, 4, D], F32, tag="yo")
                    nc.vector.tensor_mul(out=yo[:], in0=yg[:], in1=rw[:])
                    nc.scalar.dma_start(
                        out=out[ck * 512:(ck + 1) * 512, :].rearrange(
                            "(a p) d -> p a d", p=P),
                        in_=yo[:])

    nc.finalize()
    return nc


def _prepare_inputs(x, token_ids, shift_state, time_maa_k, time_maa_r,
                    w_recept, w_key, w_value):
    cfg, idxs = _build_indices(token_ids)
    x = np.asarray(x, np.float32)
    shift = np.asarray(shift_state, np.float32)
    wrt = np.ascontiguousarray(np.asarray(w_recept, np.float32).T).astype(nbf16)
    wkb = np.asarray(w_key, np.float32).astype(nbf16)
    wvb = np.asarray(w_value, np.float32).astype(nbf16)
    mk = np.asarray(time_maa_k, np.float32)[None, :].astype(nbf16)
    mr = np.asarray(time_maa_r, np.float32)[None, :].astype(nbf16)
    iota = np.tile(np.arange(T, dtype=np.int16).reshape(-1, 16).T, (8, 1))

    in_maps = []
    for k in range(NCORES):
        x_ext = np.concatenate([shift[k:k + 1], x[k]], axis=0)
        in_maps.append({
            "x_ext": np.ascontiguousarray(x_ext),
            "maa_k": mk, "maa_r": mr, "wrt": wrt,
            "wk": np.ascontiguousarray(wkb[EPC * k:EPC * (k + 1)]),
            "wv": np.ascontiguousarray(wvb[EPC * k:EPC * (k + 1)]),
            "iota16": iota,
            **idxs[k],
        })
    return cfg, in_maps


def kernel(x, token_ids, shift_state, time_maa_k, time_maa_r,
           w_recept, w_key, w_value, _trace=False):
    cfg, in_maps = _prepare_inputs(x, token_ids, shift_state, time_maa_k,
                                   time_maa_r, w_recept, w_key, w_value)
    if cfg not in _CACHE:
        _CACHE[cfg] = _build_nc(cfg)
    nc = _CACHE[cfg]
    res = run_bass_kernel_spmd(nc, in_maps, core_ids=list(range(NCORES)),
                               trace=_trace)
    kernel.last_result = res
    y = np.stack([res.results[k]["out"] for k in range(NCORES)], axis=0)
    return y.astype(np.float32)



# revision 7
# speedup vs baseline: 1.1380x; 1.1380x over previous
"""Expert-parallel CMoE kernel for 8 Trainium2 NeuronCores.

Sharding (hardcoded for B=8, T=2048, D=1024, F=2048, E=16, C=1024):
  core k owns batch k (token shift, receptance, output) and experts
  {2k, 2k+1} (FFN). Hash routing is int math on token_ids, done on host;
  the resulting permutations ship to the cores as index tensors.

v1 redesign vs the 640us baseline:
  - x ships transposed (d-major, bf16) with the shift state as column 0,
    so the token-shift mix is a free-axis slice: xk/xr are computed in
    d-major with per-partition maa scalars, one DMA read of x total.
  - the receptance matmul consumes xrT straight from SBUF (no DRAM
    round trip, no iota gather); r stays resident in SBUF for phase D.
  - xk is PE-transposed (128x128 blocks via identity matmul) back to
    token-major for the dispatch scatter.
  - receptance for the second half is deferred until after the second
    dispatch AllToAll fires, so its matmuls hide the collective.
  - expert weights for the first expert prefetch during phase A on the
    sync queue; wk1/wv1 stream during expert-0 compute.
  - phase C per expert: transposing dma_gather -> FFN1 -> relu^2 ->
    FFN2 -> indirect scatter -> per-expert combine AllToAll.
  - phase D: gather y rows, multiply by SBUF-resident r, store fp32.
All matmuls bf16 with fp32 PSUM accumulation; dropped tokens and empty
expert slots route through zeroed trash rows.
"""
import sys

for _p in ("/opt/trn_rl_repo", "/root/.axon_site/_ro/trn_rl_repo"):
    if _p not in sys.path:
        sys.path.append(_p)

import numpy as np
import ml_dtypes

import concourse.bass as bass
import concourse.bacc as bacc
import concourse.mybir as mybir
import concourse.tile as tile
from concourse import masks
from concourse.bass_utils import run_bass_kernel_spmd

P = 128
B, T, D, F, E = 8, 2048, 1024, 2048, 16
N = B * T
C = max(4, N // E)          # 1024
HASH_PRIME = 5099
NCORES = 8
EPC = E // NCORES           # experts per core = 2
DC = D // P                 # 8
FC = F // P                 # 16
BF16 = mybir.dt.bfloat16
F32 = mybir.dt.float32
I16 = mybir.dt.int16
I32 = mybir.dt.int32
nbf16 = ml_dtypes.bfloat16
AF = mybir.ActivationFunctionType
ALU = mybir.AluOpType

_CACHE = {}


def _r16(v):
    return int(-(-int(v) // 16) * 16)


def _wrap16(a):
    a = np.asarray(a, np.int16)
    w = a.reshape(-1, 16).T.copy()       # j at [j%16, j//16]
    return np.tile(w, (8, 1))            # replicated across 8 Q7 cores


def _route(token_ids):
    tid = np.asarray(token_ids).reshape(N).astype(np.int64)
    e = (tid * HASH_PRIME) % E
    onehot = (e[:, None] == np.arange(E)).astype(np.int64)
    pos = onehot.cumsum(0)[np.arange(N), e] - 1
    keep = pos < C
    return e, pos, keep


def _build_indices(token_ids):
    e, pos, keep = _route(token_ids)
    src = np.arange(N) // T
    dst = e // EPC
    el = e % EPC

    # ---- dispatch: two chunks split by local token index (first/second
    # half of each batch) so the first collective fires mid-phase-A
    local_t = np.arange(N) % T
    in_a = keep & (local_t < T // 2)
    in_b = keep & (local_t >= T // 2)

    def pack(mask):
        rank = np.zeros(N, np.int64)
        cnt = np.zeros((NCORES, NCORES), np.int64)
        for n in np.nonzero(mask)[0]:
            rank[n] = cnt[src[n], dst[n]]
            cnt[src[n], dst[n]] += 1
        return rank, _r16(cnt.max())

    rank_a, Ka = pack(in_a)
    rank_b, Kb = pack(in_b)
    srcA = np.where(in_a, dst * Ka + rank_a, NCORES * Ka)
    srcB = np.where(in_b, dst * Kb + rank_b, NCORES * Kb)

    # recv-side: slot (core d, el, c) -> row in combined recv1
    # [0 : 8Ka) chunk a, [8Ka : 8Ka+8Kb) chunk b, last row zero
    ZR1 = NCORES * (Ka + Kb)
    recv_row = np.full((NCORES, EPC * C), ZR1, np.int64)
    for n in np.nonzero(in_a)[0]:
        recv_row[dst[n], el[n] * C + pos[n]] = src[n] * Ka + rank_a[n]
    for n in np.nonzero(in_b)[0]:
        recv_row[dst[n], el[n] * C + pos[n]] = \
            NCORES * Ka + src[n] * Kb + rank_b[n]

    # ---- combine: one chunk per expert parity (el)
    rank_0, K0 = pack(keep & (el == 0))
    rank_1, K1 = pack(keep & (el == 1))
    ZROW = NCORES * (K0 + K1)
    sl2 = np.zeros((NCORES, EPC, C), np.int64)
    sl2[:, 0, :] = NCORES * K0
    sl2[:, 1, :] = NCORES * K1
    ygather = np.full(N, ZROW, np.int64)
    for n in range(N):
        if not keep[n]:
            continue
        if el[n] == 0:
            sl2[dst[n], 0, pos[n]] = src[n] * K0 + rank_0[n]
            ygather[n] = dst[n] * K0 + rank_0[n]
        else:
            sl2[dst[n], 1, pos[n]] = src[n] * K1 + rank_1[n]
            ygather[n] = NCORES * K0 + dst[n] * K1 + rank_1[n]

    per_core = []
    for k in range(NCORES):
        tok = slice(k * T, (k + 1) * T)
        # dispatch offsets: col t<8 indexes a1a (srcA), t>=8 a1b (srcB)
        src32 = np.concatenate(
            [srcA[tok].astype(np.int32).reshape(T // P, P)[:8],
             srcB[tok].astype(np.int32).reshape(T // P, P)[8:]],
            axis=0).T.copy()
        per_core.append({
            "src32": src32,
            "slot16": _wrap16(recv_row[k]),
            "sl2_32": np.concatenate(
                [sl2[k, 0].reshape(-1, P).T, sl2[k, 1].reshape(-1, P).T],
                axis=1).astype(np.int32).copy(),
            "ygather16": _wrap16(ygather[tok]),
        })
    return (Ka, Kb, K0, K1), per_core


def _build_nc(cfg):
    Ka, Kb, K0, K1 = cfg
    K2 = {0: K0, 1: K1}
    off2 = {0: 0, 1: NCORES * K0}
    R2 = NCORES * (K0 + K1)
    nc = bacc.Bacc("TRN2", target_bir_lowering=False, debug=False,
                   num_devices=NCORES)

    x_t = nc.dram_tensor("x_t", [P, DC, T + 1], BF16, kind="ExternalInput")
    maa2 = nc.dram_tensor("maa2", [P, 2 * DC], BF16, kind="ExternalInput")
    wrt = nc.dram_tensor("wrt", [D, D], BF16, kind="ExternalInput")
    wk = nc.dram_tensor("wk", [EPC, D, F], BF16, kind="ExternalInput")
    wv = nc.dram_tensor("wv", [EPC, F, D], BF16, kind="ExternalInput")
    src32 = nc.dram_tensor("src32", [P, T // P], I32, kind="ExternalInput")
    slot16 = nc.dram_tensor("slot16", [P, EPC * C // 16], I16,
                            kind="ExternalInput")
    sl2_32 = nc.dram_tensor("sl2_32", [P, EPC * C // P], I32,
                            kind="ExternalInput")
    ygather16 = nc.dram_tensor("ygather16", [P, T // 16], I16,
                               kind="ExternalInput")
    out = nc.dram_tensor("out", [T, D], F32, kind="ExternalOutput")

    rg = [list(range(NCORES))]

    with tile.TileContext(nc) as tc:
        with (
            tc.tile_pool(name="dram", bufs=1, space="DRAM") as dram,
            tc.tile_pool(name="misc", bufs=1) as misc,
            tc.tile_pool(name="pwk", bufs=1) as pwk,
            tc.tile_pool(name="pwv", bufs=1) as pwv,
        ):
            a1a_in = dram.tile([NCORES * Ka + 1, D], BF16)
            a1b_in = dram.tile([NCORES * Kb + 1, D], BF16)
            recv1 = dram.tile([NCORES * (Ka + Kb) + 1, D], BF16)
            a2_in = {eli: dram.tile([NCORES * K2[eli] + 1, D], BF16,
                                    name=f"a2in_{eli}")
                     for eli in range(EPC)}
            recv2 = dram.tile([R2 + 1, D], BF16)

            zrow = misc.tile([1, D], BF16)
            nc.vector.memzero(zrow[:])
            nc.sync.dma_start(
                out=recv1[NCORES * (Ka + Kb):NCORES * (Ka + Kb) + 1, :],
                in_=zrow[:])
            nc.sync.dma_start(out=recv2[R2:R2 + 1, :], in_=zrow[:])

            # small index tensors first on the sync queue
            m2 = misc.tile([P, 2 * DC], BF16)
            nc.sync.dma_start(out=m2[:], in_=maa2[:])
            s32 = misc.tile([P, T // P], I32)
            nc.sync.dma_start(out=s32[:], in_=src32[:])
            sl16 = misc.tile([P, EPC * C // 16], I16)
            nc.sync.dma_start(out=sl16[:], in_=slot16[:])
            s232 = misc.tile([P, EPC * C // P], I32)
            nc.sync.dma_start(out=s232[:], in_=sl2_32[:])
            yg16 = misc.tile([P, T // 16], I16)
            nc.sync.dma_start(out=yg16[:], in_=ygather16[:])

            ident = misc.tile([P, P], BF16)
            masks.make_identity(nc, ident[:])

            # receptance output stays resident in SBUF for phase D
            r_sb = misc.tile([P, T // P, D], BF16)

            wrt_sb = misc.tile([P, DC, D], BF16)
            wk_sb = {}
            wv_sb = {}

            def recept(hidx, xr_h, psr):
                """receptance for 512 tokens: r = sigmoid(xr @ wrt)"""
                for tt in range(4):
                    pr0 = psr.tile([P, 512], F32, space="PSUM", tag="pr0")
                    pr1 = psr.tile([P, 512], F32, space="PSUM", tag="pr1")
                    for dc in range(DC):
                        nc.tensor.matmul(
                            out=pr0[:],
                            lhsT=xr_h[:, dc, tt * P:(tt + 1) * P],
                            rhs=wrt_sb[:, dc, 0:512],
                            start=(dc == 0), stop=(dc == DC - 1))
                        nc.tensor.matmul(
                            out=pr1[:],
                            lhsT=xr_h[:, dc, tt * P:(tt + 1) * P],
                            rhs=wrt_sb[:, dc, 512:1024],
                            start=(dc == 0), stop=(dc == DC - 1))
                    ti = hidx * 4 + tt
                    nc.scalar.activation(out=r_sb[:, ti, 0:512], in_=pr0[:],
                                         func=AF.Sigmoid)
                    nc.scalar.activation(out=r_sb[:, ti, 512:1024],
                                         in_=pr1[:], func=AF.Sigmoid)

            with tc.tile_pool(name="pxr", bufs=3) as pxr, \
                 tc.tile_pool(name="psr", bufs=2, space="PSUM") as psr:
                xr_saved = {}
                # ---------------- phase A: token-shift mix in d-major
                with (
                    tc.tile_pool(name="pxb", bufs=2) as pxb,
                    tc.tile_pool(name="pdx", bufs=1) as pdx,
                    tc.tile_pool(name="pxk", bufs=2) as pxk,
                    tc.tile_pool(name="pxtm", bufs=2) as pxtm,
                    tc.tile_pool(name="pst", bufs=2, space="PSUM") as pst,
                ):
                    xbufs = []
                    for ck in range(2):
                        xb = pxb.tile([P, DC, 1025], BF16, tag="xb")
                        nc.sync.dma_start(
                            out=xb[:],
                            in_=x_t[:, :, ck * 1024:ck * 1024 + 1025])
                        xbufs.append(xb)
                        if ck == 0:
                            # big weight prefetches ride sync behind x
                            nc.sync.dma_start(
                                out=wrt_sb[:],
                                in_=wrt.rearrange("(c p) e -> p c e", p=P))
                    for el in range(EPC):
                        wk_sb[el] = pwk.tile([P, DC, F], BF16, tag="wk", name=f"wk_sb{el}")
                        wv_sb[el] = pwv.tile([P, FC, D], BF16, tag="wv", name=f"wv_sb{el}")
                        if el == 0:
                            nc.sync.dma_start(
                                out=wk_sb[0][:],
                                in_=wk[0].rearrange("(c p) f -> p c f", p=P))
                            nc.sync.dma_start(
                                out=wv_sb[0][:],
                                in_=wv[0].rearrange("(c p) f -> p c f", p=P))

                    for ck in range(2):
                        xb = xbufs[ck]
                        for h in range(2):
                            hidx = ck * 2 + h
                            o = h * 512
                            dx = pdx.tile([P, DC, 512], BF16, tag="dx")
                            # dxprev = xprev - x
                            nc.vector.tensor_sub(
                                out=dx[:], in0=xb[:, :, o:o + 512],
                                in1=xb[:, :, o + 1:o + 513])
                            xk_h = pxk.tile([P, DC, 512], BF16, tag="xk")
                            xr_h = pxr.tile([P, DC, 512], BF16, tag="xr")
                            for c in range(DC):
                                nc.vector.scalar_tensor_tensor(
                                    out=xk_h[:, c, :], in0=dx[:, c, :],
                                    scalar=m2[:, c:c + 1],
                                    in1=xb[:, c, o + 1:o + 513],
                                    op0=ALU.mult, op1=ALU.add)
                                nc.vector.scalar_tensor_tensor(
                                    out=xr_h[:, c, :], in0=dx[:, c, :],
                                    scalar=m2[:, DC + c:DC + c + 1],
                                    in1=xb[:, c, o + 1:o + 513],
                                    op0=ALU.mult, op1=ALU.add)
                            # transpose xk to token-major, scatter rows
                            for tt in range(4):
                                ps = pst.tile([P, DC, P], BF16, space="PSUM",
                                              tag="ps")
                                for c in range(DC):
                                    nc.tensor.transpose(
                                        out=ps[:, c, :],
                                        in_=xk_h[:, c, tt * P:(tt + 1) * P],
                                        identity=ident[:])
                                xtm = pxtm.tile([P, D], BF16, tag="xtm")
                                nc.vector.tensor_copy(out=xtm[:], in_=ps[:])
                                t = hidx * 4 + tt
                                buf = a1a_in if ck == 0 else a1b_in
                                nc.gpsimd.indirect_dma_start(
                                    out=buf[:],
                                    out_offset=bass.IndirectOffsetOnAxis(
                                        ap=s32[:, t:t + 1], axis=0),
                                    in_=xtm[:], in_offset=None)
                            if hidx < 2:
                                recept(hidx, xr_h, psr)
                            else:
                                xr_saved[hidx] = xr_h
                        if ck == 0:
                            nc.gpsimd.collective_compute(
                                "AllToAll", mybir.AluOpType.bypass,
                                replica_groups=rg,
                                ins=[a1a_in[0:NCORES * Ka, :]],
                                outs=[recv1[0:NCORES * Ka, :]])
                    nc.gpsimd.collective_compute(
                        "AllToAll", mybir.AluOpType.bypass, replica_groups=rg,
                        ins=[a1b_in[0:NCORES * Kb, :]],
                        outs=[recv1[NCORES * Ka:NCORES * (Ka + Kb), :]])

                # deferred receptance hides the second dispatch AllToAll
                recept(2, xr_saved[2], psr)
                recept(3, xr_saved[3], psr)

            # ---------------- phase C: expert FFNs
            with (
                tc.tile_pool(name="pfx", bufs=2) as pfx,
                tc.tile_pool(name="pfh", bufs=1) as pfh,
                tc.tile_pool(name="phr", bufs=2) as phr,
                tc.tile_pool(name="pfy", bufs=3) as pfy,
                tc.tile_pool(name="psh", bufs=2, space="PSUM") as psh,
                tc.tile_pool(name="psy", bufs=2, space="PSUM") as psy,
            ):
                for el in range(EPC):
                    if el == 1:
                        nc.sync.dma_start(
                            out=wk_sb[1][:],
                            in_=wk[1].rearrange("(c p) f -> p c f", p=P))
                        nc.sync.dma_start(
                            out=wv_sb[1][:],
                            in_=wv[1].rearrange("(c p) f -> p c f", p=P))
                    for ck in range(2):
                        XT = pfx.tile([P, DC, 512], BF16, tag="XT")
                        col0 = (el * C + ck * 512) // 16
                        nc.gpsimd.dma_gather(
                            out_ap=XT[:], in_ap=recv1[:],
                            idxs_ap=sl16[:, col0:col0 + 32],
                            num_idxs=512, num_idxs_reg=512, elem_size=D,
                            transpose=True)
                        ht = pfh.tile([P, FC, 512], BF16, tag="ht")
                        for ft in range(FC):
                            ph = psh.tile([P, 512], F32, space="PSUM", tag="ph")
                            for dc in range(DC):
                                nc.tensor.matmul(
                                    out=ph[:],
                                    lhsT=wk_sb[el][:, dc, ft * P:(ft + 1) * P],
                                    rhs=XT[:, dc, :],
                                    start=(dc == 0), stop=(dc == DC - 1))
                            hr = phr.tile([P, 512], BF16, tag="hr")
                            nc.scalar.activation(out=hr[:], in_=ph[:],
                                                 func=AF.Relu)
                            nc.vector.tensor_mul(out=ht[:, ft, :], in0=hr[:],
                                                 in1=hr[:])
                        for tt in range(4):
                            ysb = pfy.tile([P, D], BF16, tag="ysb")
                            py0 = psy.tile([P, 512], F32, space="PSUM", tag="py0")
                            py1 = psy.tile([P, 512], F32, space="PSUM", tag="py1")
                            for fc in range(FC):
                                nc.tensor.matmul(
                                    out=py0[:],
                                    lhsT=ht[:, fc, tt * P:(tt + 1) * P],
                                    rhs=wv_sb[el][:, fc, 0:512],
                                    start=(fc == 0), stop=(fc == FC - 1))
                                nc.tensor.matmul(
                                    out=py1[:],
                                    lhsT=ht[:, fc, tt * P:(tt + 1) * P],
                                    rhs=wv_sb[el][:, fc, 512:1024],
                                    start=(fc == 0), stop=(fc == FC - 1))
                            nc.vector.tensor_copy(out=ysb[:, 0:512], in_=py0[:])
                            nc.vector.tensor_copy(out=ysb[:, 512:1024],
                                                  in_=py1[:])
                            scol = el * (C // P) + ck * 4 + tt
                            nc.gpsimd.indirect_dma_start(
                                out=a2_in[el][:],
                                out_offset=bass.IndirectOffsetOnAxis(
                                    ap=s232[:, scol:scol + 1], axis=0),
                                in_=ysb[:], in_offset=None)
                    nc.gpsimd.collective_compute(
                        "AllToAll", mybir.AluOpType.bypass, replica_groups=rg,
                        ins=[a2_in[el][0:NCORES * K2[el], :]],
                        outs=[recv2[off2[el]:off2[el] + NCORES * K2[el], :]])

            # ---------------- phase D: gather own rows, multiply by r
            with tc.tile_pool(name="pd", bufs=2) as pd:
                for ck in range(T // 512):
                    yg = pd.tile([P, 4, D], BF16, tag="yg")
                    nc.gpsimd.dma_gather(
                        out_ap=yg[:], in_ap=recv2[:],
                        idxs_ap=yg16[:, ck * 32:(ck + 1) * 32],
                        num_idxs=512, num_idxs_reg=512, elem_size=D,
                        transpose=False)
                    yo = pd.tile([P, 4, D], F32, tag="yo")
                    nc.vector.tensor_mul(out=yo[:], in0=yg[:],
                                         in1=r_sb[:, ck * 4:(ck + 1) * 4, :])
                    nc.scalar.dma_start(
                        out=out[ck * 512:(ck + 1) * 512, :].rearrange(
                            "(a p) d -> p a d", p=P),
                        in_=yo[:])

    nc.finalize()
    return nc


def _prepare_inputs(x, token_ids, shift_state, time_maa_k, time_maa_r,
                    w_recept, w_key, w_value):
    cfg, idxs = _build_indices(token_ids)
    x = np.asarray(x, np.float32)
    shift = np.asarray(shift_state, np.float32)
    wrt = np.ascontiguousarray(np.asarray(w_recept, np.float32).T).astype(nbf16)
    wkb = np.asarray(w_key, np.float32).astype(nbf16)
    wvb = np.asarray(w_value, np.float32).astype(nbf16)
    mk = np.asarray(time_maa_k, np.float32)
    mr = np.asarray(time_maa_r, np.float32)
    # [P, 2*DC]: col c = maa_k[c*128+p], col DC+c = maa_r[c*128+p]
    maa2 = np.concatenate(
        [mk.reshape(DC, P).T, mr.reshape(DC, P).T], axis=1).astype(nbf16)
    maa2 = np.ascontiguousarray(maa2)

    in_maps = []
    for k in range(NCORES):
        # x transposed, d-major: x_t[p, c, 1+t] = x[k, t, c*128+p]
        xk_full = np.concatenate([shift[k][:, None], x[k].T], axis=1)
        x_t = np.ascontiguousarray(
            xk_full.reshape(DC, P, T + 1).transpose(1, 0, 2)).astype(nbf16)
        in_maps.append({
            "x_t": x_t,
            "maa2": maa2, "wrt": wrt,
            "wk": np.ascontiguousarray(wkb[EPC * k:EPC * (k + 1)]),
            "wv": np.ascontiguousarray(wvb[EPC * k:EPC * (k + 1)]),
            **idxs[k],
        })
    return cfg, in_maps


def kernel(x, token_ids, shift_state, time_maa_k, time_maa_r,
           w_recept, w_key, w_value, _trace=False):
    cfg, in_maps = _prepare_inputs(x, token_ids, shift_state, time_maa_k,
                                   time_maa_r, w_recept, w_key, w_value)
    if cfg not in _CACHE:
        _CACHE[cfg] = _build_nc(cfg)
    nc = _CACHE[cfg]
    res = run_bass_kernel_spmd(nc, in_maps, core_ids=list(range(NCORES)),
                               trace=_trace)
    kernel.last_result = res
    y = np.stack([res.results[k]["out"] for k in range(NCORES)], axis=0)
    return y.astype(np.float32)
